# revision 57
# baseline (speedup 1.0000x reference)
"""LocalTransformerEncoderLayer on 8 trn2 NeuronCores.

Sharding: core c = 2*b + h handles batch b, sequence half h (4096 tokens,
plus a 64-token halo on each side for the local-attention window).
Everything is done on-device per core; no collectives needed.

v2 design (all fp16, LN-scale-invariance, deep software pipeline):
  srcT  [512, 4224] fp16  d-major haloed chunk (host-transposed)
  qT,kT [128,4,4224] fp16 d-major in SBUF (PE: W.T @ srcT, bias via ACT)
  v     -> DRAM scratch [33*128, 512] fp16 (token-major tiles)
  per q-pair p (128 queries, 256 keys = ext tiles p,p+1):
    simT [128k, 256q... actually [128keys, 2*128q]] psum = kT.T @ qT + rank-1 masks
    expT fp16 = ACT exp(scale*sim); den[128,1] psum via ones-matmul (shares bank)
    s' = den*src + expT.T@v   (LN is scale-invariant per token -> skip 1/den)
    LN1 stats on s' (DVE bn_stats fp16); x fp16 = ACT(s'*rstd - m*rstd)
  xT via PE transpose; FFN h=relu(W1.T@xT) fp16; y = h.T@W2; s2 = x + y
  LN2 per tile (unbatched, pipelined tail), out fp16 DMA, host casts f32.
Pipeline lags (pair steps): ln1 @+5, transposes @+8, ffn_h @+10, ffn_y @+11.
"""
import os
import numpy as np

_F16 = np.float16

B, N, D, F, W = 4, 8192, 512, 2048, 64
T = N // 2            # own tokens per core = 4096
H = 64                # halo
TEXT = T + 2 * H      # 4224
NPAIR = T // 128      # 32 q-pairs per core
NBLK = T // 512       # 8 blocks
NEG = -60000.0        # fp16-safe mask value (exp(scale*(-60000+eps)) == 0)
SCALE = float(D) ** -0.5

_cache = {}


def _build(apply_bv, apply_b2, apply_ln1g, apply_ln1b, apply_ln2g, apply_ln2b,
           apply_b1, apply_bqk):
    # Fast path: when the FFN is bias-free and LN1 has trivial gamma/beta,
    # LN1's rstd never needs applying: relu commutes with positive per-token
    # scales and LN2 is scale-invariant, so mean-centering s' suffices.
    fast = not (apply_ln1g or apply_ln1b or apply_b1 or apply_b2)
    # Merged QK: with bq=bk=0, sim = (src Wq)(src Wk)^T = (src M) src^T with
    # M = Wq Wk^T host-precomputed ("wq" input) -> the K projection vanishes
    # and the d-major srcT tiles serve directly as the key matrix.
    merged = not apply_bqk
    import concourse.bacc as bacc
    import concourse.tile as tile
    from concourse import mybir
    import concourse.bass as bass

    f32 = mybir.dt.float32
    f16 = mybir.dt.float16
    AF = mybir.ActivationFunctionType
    ALU = mybir.AluOpType

    nc = bacc.Bacc("TRN2", target_bir_lowering=False, debug=False)

    # ---- DRAM I/O ----
    srcT_d = nc.dram_tensor("srcT", [D, TEXT], f16, kind="ExternalInput").ap()
    src_d = nc.dram_tensor("src", [T, D], f16, kind="ExternalInput").ap()
    wq_d = nc.dram_tensor("wq", [D, D], f16, kind="ExternalInput").ap()
    if not apply_bqk:
        wk_d = None
        bqT_d = bkT_d = None
    else:
        wk_d = nc.dram_tensor("wk", [D, D], f16, kind="ExternalInput").ap()
        bqT_d = nc.dram_tensor("bqT", [128, 4], f32, kind="ExternalInput").ap()
        bkT_d = nc.dram_tensor("bkT", [128, 4], f32, kind="ExternalInput").ap()
    wv_d = nc.dram_tensor("wv", [D, D], f16, kind="ExternalInput").ap()
    w1_d = nc.dram_tensor("w1", [D, F], f16, kind="ExternalInput").ap()
    b1T_d = nc.dram_tensor("b1T", [128, 16], f32, kind="ExternalInput").ap()
    w2_d = nc.dram_tensor("w2", [F, D], f16, kind="ExternalInput").ap()
    ident_d = nc.dram_tensor("ident", [128, 128], f16, kind="ExternalInput").ap()
    uA_d = nc.dram_tensor("uA", [1, 128], f16, kind="ExternalInput").ap()
    uB_d = nc.dram_tensor("uB", [1, 128], f16, kind="ExternalInput").ap()
    wA_d = nc.dram_tensor("wA", [1, 128], f16, kind="ExternalInput").ap()
    wB_d = nc.dram_tensor("wB", [1, 128], f16, kind="ExternalInput").ap()
    wA0_d = nc.dram_tensor("wA0", [1, 128], f16, kind="ExternalInput").ap()
    wB31_d = nc.dram_tensor("wB31", [1, 128], f16, kind="ExternalInput").ap()
    onerow_d = nc.dram_tensor("onerow", [1, 128], f16, kind="ExternalInput").ap()
    if apply_bv:
        bvrow_d = nc.dram_tensor("bvrow", [1, D], f16, kind="ExternalInput").ap()
    if apply_b2:
        b2row_d = nc.dram_tensor("b2row", [1, D], f16, kind="ExternalInput").ap()
    if apply_ln1g:
        g1_d = nc.dram_tensor("g1", [128, D], f16, kind="ExternalInput").ap()
    if apply_ln1b:
        be1_d = nc.dram_tensor("be1", [128, D], f16, kind="ExternalInput").ap()
    if apply_ln2g:
        g2_d = nc.dram_tensor("g2", [128, D], f16, kind="ExternalInput").ap()
    if apply_ln2b:
        be2_d = nc.dram_tensor("be2", [128, D], f16, kind="ExternalInput").ap()
    out_d = nc.dram_tensor("out", [T, D], f16, kind="ExternalOutput").ap()

    from contextlib import ExitStack
    with tile.TileContext(nc) as tc, ExitStack() as ctx:
        # ---- persistent pools ----
        consts = ctx.enter_context(tc.tile_pool(name="consts", bufs=1))
        kv = ctx.enter_context(tc.tile_pool(name="kv", bufs=1))
        big_ps = ctx.enter_context(tc.tile_pool(name="big_ps", bufs=2, space="PSUM"))
        av_ps = ctx.enter_context(tc.tile_pool(name="av_ps", bufs=2, space="PSUM"))
        sim_ps = ctx.enter_context(tc.tile_pool(name="sim_ps", bufs=2, space="PSUM"))
        xt_ps = ctx.enter_context(tc.tile_pool(name="xt_ps", bufs=2, space="PSUM"))

        # constants: srcT block 0 + wq first so the first matmuls start ASAP
        # srcT lives in per-block persistent tiles (doubles as the key matrix
        # in merged-QK mode; 128-col key tiles never straddle a 512 block)
        nsrct = TEXT // 512 + 1
        srcT_r = srcT_d.rearrange("(dt p) t -> p dt t", p=128)
        srcT_tiles = [consts.tile([128, 4, 512], f16, tag=f"srcT{i}",
                                  name=f"srcTp{i}")
                      for i in range(nsrct)]
        wq_sb = consts.tile([128, 4, D], f16, tag="wq")
        wq_r = wq_d.rearrange("(kt p) m -> p kt m", p=128)
        for kt in range(4):
            nc.sync.dma_start(srcT_tiles[0][:, kt, :], srcT_r[:, kt, 0:512])
            nc.sync.dma_start(wq_sb[:, kt, :], wq_r[:, kt, :])
        for i in range(1, 3):
            nc.sync.dma_start(srcT_tiles[i], srcT_r[:, :, i * 512:(i + 1) * 512])
        if not merged:
            wk_sb = consts.tile([128, 4, D], f16, tag="wk")
            nc.scalar.dma_start(wk_sb, wk_d.rearrange("(kt p) m -> p kt m", p=128))
        wv_sb = consts.tile([128, 4, D], f16, tag="wv")
        nc.scalar.dma_start(wv_sb, wv_d.rearrange("(kt p) m -> p kt m", p=128))
        w1_sb = consts.tile([128, 4, F], f16, tag="w1")
        w2_sb = consts.tile([128, 16, D], f16, tag="w2")
        if not merged:
            bqT_sb = consts.tile([128, 4], f32, tag="bqT")
            nc.sync.dma_start(bqT_sb, bqT_d)
            bkT_sb = consts.tile([128, 4], f32, tag="bkT")
            nc.sync.dma_start(bkT_sb, bkT_d)
        b1T_sb = consts.tile([128, 16], f32, tag="b1T")
        nc.sync.dma_start(b1T_sb, b1T_d)
        # small phase-2 constants early (cheap; needed by sim(0) which the
        # scheduler may hoist into phase 1)
        ident_sb = consts.tile([128, 128], f16, tag="ident")
        nc.scalar.dma_start(ident_sb, ident_d)
        uA_sb = consts.tile([1, 128], f16, tag="uA")
        nc.scalar.dma_start(uA_sb, uA_d)
        uB_sb = consts.tile([1, 128], f16, tag="uB")
        nc.scalar.dma_start(uB_sb, uB_d)
        wA_sb = consts.tile([1, 128], f16, tag="wA")
        nc.scalar.dma_start(wA_sb, wA_d)
        wB_sb = consts.tile([1, 128], f16, tag="wB")
        nc.scalar.dma_start(wB_sb, wB_d)
        wA0_sb = consts.tile([1, 128], f16, tag="wA0")
        nc.scalar.dma_start(wA0_sb, wA0_d)
        wB31_sb = consts.tile([1, 128], f16, tag="wB31")
        nc.scalar.dma_start(wB31_sb, wB31_d)
        ones_sb = consts.tile([128, 1], f16, tag="ones")
        nc.vector.memset(ones_sb, 1.0)
        eps_sb = consts.tile([128, 1], f32, tag="eps")
        nc.vector.memset(eps_sb, 1e-5)
        # warm the ACT table (exp_and_others covers Exp/Identity/Copy/Relu)
        # during the DMA prefix instead of at the first real activation
        actwarm_sb = consts.tile([128, 1], f32, tag="actwarm")
        nc.scalar.activation(actwarm_sb, eps_sb, AF.Exp)
        if apply_bv:
            onerow_sb = consts.tile([1, 128], f16, tag="onerow")
            nc.sync.dma_start(onerow_sb, onerow_d)
            bvrow_sb = consts.tile([1, D], f16, tag="bvrow")
            nc.sync.dma_start(bvrow_sb, bvrow_d)
        if apply_b2:
            onerow2_sb = consts.tile([1, 128], f16, tag="onerow2")
            nc.sync.dma_start(onerow2_sb, onerow_d)
            b2row_sb = consts.tile([1, D], f16, tag="b2row")
            nc.sync.dma_start(b2row_sb, b2row_d)
        if apply_ln1g:
            g1_sb = consts.tile([128, D], f16, tag="g1")
            nc.sync.dma_start(g1_sb, g1_d)
        if apply_ln1b:
            be1_sb = consts.tile([128, D], f16, tag="be1")
            nc.sync.dma_start(be1_sb, be1_d)
        if apply_ln2g:
            g2_sb = consts.tile([128, D], f16, tag="g2")
            nc.sync.dma_start(g2_sb, g2_d)
        if apply_ln2b:
            be2_sb = consts.tile([128, D], f16, tag="be2")
            nc.sync.dma_start(be2_sb, be2_d)

        # persistent activations
        qT_sb = kv.tile([128, 4, TEXT], f16, tag="qT")
        v_tiles = [kv.tile([128, D], f16, tag=f"v{i}", name=f"vt{i}")
                   for i in range(33)]
        if not merged:
            kT_sb = kv.tile([128, 4, TEXT], f16, tag="kT")

        # ---- phase 1: QKV over ext grid (srcT streamed per block) ----
        blocks = [(i * 512, 512) for i in range(TEXT // 512)] + [(4096, 128)]
        for bi, (off, tw) in enumerate(blocks):
            srcT_sb = srcT_tiles[bi]
            if bi + 2 < nsrct:
                lo2, hi2 = (bi + 2) * 512, min((bi + 3) * 512, TEXT)
                nc.sync.dma_start(srcT_tiles[bi + 2][:, :, :hi2 - lo2],
                                  srcT_r[:, :, lo2:hi2])
            # qT (and kT if not merged), d-major
            projs = [(wq_sb, bqT_sb if not merged else None, qT_sb)]
            if not merged:
                projs.append((wk_sb, bkT_sb, kT_sb))
            for w_sb, b_sb, dst in projs:
                for dq in range(4):
                    ps = big_ps.tile([128, 512], f32, tag="big")
                    for kt in range(4):
                        nc.tensor.matmul(
                            ps[:, :tw],
                            lhsT=w_sb[:, kt, dq * 128:(dq + 1) * 128],
                            rhs=srcT_sb[:, kt, :tw],
                            start=(kt == 0), stop=(kt == 3),
                        )
                    if b_sb is not None:
                        nc.scalar.activation(
                            dst[:, dq, off:off + tw], ps[:, :tw],
                            AF.Identity, bias=b_sb[:, dq:dq + 1],
                        )
                    else:
                        nc.scalar.copy(dst[:, dq, off:off + tw], ps[:, :tw])
            # v (token-major), per 128-token tile -> DRAM scratch
            for s in range(tw // 128):
                ti = (off + s * 128) // 128
                ps = big_ps.tile([128, 512], f32, tag="big")
                for kt in range(4):
                    nc.tensor.matmul(
                        ps,
                        lhsT=srcT_sb[:, kt, s * 128:s * 128 + 128],
                        rhs=wv_sb[:, kt, :],
                        start=(kt == 0), stop=(kt == 3 and not apply_bv),
                    )
                if apply_bv:
                    nc.tensor.matmul(ps, lhsT=onerow_sb, rhs=bvrow_sb,
                                     start=False, stop=True)
                nc.vector.tensor_copy(v_tiles[ti], ps)
            # kick FFN weight DMAs while late phase-1 blocks still compute
            if bi == 5:
                nc.scalar.dma_start(w1_sb, w1_d.rearrange("(kt p) m -> p kt m", p=128))
            if bi == 7:
                nc.scalar.dma_start(w2_sb, w2_d.rearrange("(ft p) m -> p ft m", p=128))

        # ---- phase 2 pools ----
        x_pool = ctx.enter_context(tc.tile_pool(name="x_pool", bufs=8))
        xT_pool = ctx.enter_context(tc.tile_pool(name="xT_pool", bufs=2))
        h_pool = ctx.enter_context(tc.tile_pool(name="h_pool", bufs=1))
        attn_pool = ctx.enter_context(tc.tile_pool(name="attn_pool", bufs=2))
        io_pool = ctx.enter_context(tc.tile_pool(name="io_pool", bufs=3))
        stat_pool = ctx.enter_context(tc.tile_pool(name="stat_pool", bufs=8))
        exp_pool = ctx.enter_context(tc.tile_pool(name="exp_pool", bufs=4))
        res_pool = ctx.enter_context(tc.tile_pool(name="res_pool", bufs=4))
        den_pool = ctx.enter_context(tc.tile_pool(name="den_pool", bufs=12))

        # ---- phase 2 state ----
        expT_t = {}
        sim_t = {}
        src_t_t = {}
        s_tiles = {}
        mv1 = {}
        x_tiles = {}
        xT_blks = {}
        h_blks = {}
        den_tiles = {}

        def emit_sim(p):
            qoff = H + p * 128
            src_t = io_pool.tile([128, D], f16, tag="srct")
            nc.sync.dma_start(src_t, src_d[p * 128:(p + 1) * 128, :])
            src_t_t[p] = src_t
            # sim: [128 keys, 2*128 queries] (+1 col for denominator later)
            ps_sim = sim_ps.tile([128, 260], f32, tag="sim")
            sim_t[p] = ps_sim
            for half, (ktile, u_sb, w_vec) in enumerate((
                (p, uA_sb, wA0_sb if p == 0 else wA_sb),
                (p + 1, uB_sb, wB31_sb if p == NPAIR - 1 else wB_sb),
            )):
                reg = ps_sim[:, half * 128:(half + 1) * 128]
                if merged:
                    kt_sb = srcT_tiles[ktile * 128 // 512]
                    kcol = (ktile * 128) % 512
                else:
                    kt_sb, kcol = kT_sb, ktile * 128
                for kt in range(4):
                    nc.tensor.matmul(
                        reg,
                        lhsT=kt_sb[:, kt, kcol:kcol + 128],
                        rhs=qT_sb[:, kt, qoff:qoff + 128],
                        start=(kt == 0), stop=False,
                    )
                nc.tensor.matmul(reg, lhsT=u_sb, rhs=w_vec, start=False, stop=True)
            expT = exp_pool.tile([128, 256], f16, tag="expT")
            nc.scalar.activation(expT, ps_sim[:, 0:256], AF.Exp, scale=SCALE)
            expT_t[p] = expT

        def emit_av(p):
            expT = expT_t.pop(p)
            vA, vB = v_tiles[p], v_tiles[p + 1]
            ps_sim = sim_t.pop(p)
            ps_den = ps_sim[:, 256:257]
            nc.tensor.matmul(ps_den, lhsT=expT[:, 0:128], rhs=ones_sb,
                             start=True, stop=False)
            nc.tensor.matmul(ps_den, lhsT=expT[:, 128:256], rhs=ones_sb,
                             start=False, stop=True)
            ps_av = av_ps.tile([128, 512], f32, tag="av")
            nc.tensor.matmul(ps_av, lhsT=expT[:, 0:128], rhs=vA,
                             start=True, stop=False)
            nc.tensor.matmul(ps_av, lhsT=expT[:, 128:256], rhs=vB,
                             start=False, stop=True)
            den_sb = den_pool.tile([128, 1], f32, tag="den")
            nc.scalar.copy(den_sb, ps_den)
            den_tiles[p] = den_sb
            # s' = den*src + av  (== den * (src + attn); LN1 is scale-invariant)
            tmp = attn_pool.tile([128, D], f16, tag="tmp")
            nc.scalar.activation(tmp, src_t_t.pop(p), AF.Identity, scale=den_sb)
            s_sb = res_pool.tile([128, D], f16, tag="s")
            nc.vector.tensor_add(s_sb, tmp, ps_av)
            if fast:
                # z = s' - mean(s'): the den and rstd1 scales wash out in LN2
                nsum = stat_pool.tile([128, 1], f32, tag="nsum")
                nc.vector.reduce_sum(nsum, s_sb, axis=mybir.AxisListType.X)
                nmean = stat_pool.tile([128, 1], f32, tag="nmean")
                nc.vector.tensor_scalar(nmean, nsum, -1.0 / D, None, ALU.mult)
                x_t = x_pool.tile([128, D], f16, tag="x")
                nc.vector.tensor_scalar(x_t, s_sb, nmean, None, ALU.add)
                x_tiles[p] = x_t
            else:
                s_tiles[p] = s_sb
                st6 = stat_pool.tile([128, 6], f32, tag="st6")
                nc.vector.bn_stats(st6, s_sb)
                mv = stat_pool.tile([128, 2], f32, tag="mv1")
                nc.vector.bn_aggr(mv, st6)
                mv1[p] = mv

        def ln_compute(blk):
            """full-path LN1: rstd via DVE pow, x via scalar activation"""
            mvs = [mv1.pop(blk * 4 + j) for j in range(4)]
            var_blk = stat_pool.tile([128, 4], f32, tag="b1v")
            mean_blk = stat_pool.tile([128, 4], f32, tag="b1m")
            for j, mv in enumerate(mvs):
                nc.gpsimd.tensor_copy(var_blk[:, j:j + 1], mv[:, 1:2])
                nc.gpsimd.tensor_copy(mean_blk[:, j:j + 1], mv[:, 0:1])
            lnv_blk = stat_pool.tile([128, 4], f32, tag="b1l")
            nc.scalar.activation(lnv_blk, var_blk, AF.Ln, bias=eps_sb)
            rstd_blk = stat_pool.tile([128, 4], f32, tag="b1r")
            nc.scalar.activation(rstd_blk, lnv_blk, AF.Exp, scale=-0.5)
            nmr_blk = stat_pool.tile([128, 4], f32, tag="b1n")
            nc.vector.tensor_scalar(nmr_blk, mean_blk, -1.0, None, ALU.mult)
            nc.vector.tensor_mul(nmr_blk, nmr_blk, rstd_blk)
            for j in range(4):
                p = blk * 4 + j
                x_t = x_pool.tile([128, D], f16, tag="x")
                nc.scalar.activation(x_t, s_tiles.pop(p), AF.Identity,
                                     scale=rstd_blk[:, j:j + 1],
                                     bias=nmr_blk[:, j:j + 1])
                if apply_ln1g:
                    nc.vector.tensor_mul(x_t, x_t, g1_sb)
                if apply_ln1b:
                    nc.vector.tensor_add(x_t, x_t, be1_sb)
                x_tiles[p] = x_t

        def emit_transposes(blk):
            xT_blk = xT_pool.tile([128, 4, 512], f16, tag="xT")
            xT_blks[blk] = xT_blk
            for j in range(4):
                x_t = x_tiles[blk * 4 + j]
                for dt in range(4):
                    ps_xt = xt_ps.tile([128, 128], f16, tag="xt")
                    nc.tensor.transpose(ps_xt, x_t[:, dt * 128:(dt + 1) * 128],
                                        ident_sb)
                    dst = xT_blk[:, dt, j * 128:(j + 1) * 128]
                    if (j * 4 + dt) % 2 == 0:
                        nc.vector.tensor_copy(dst, ps_xt)
                    else:
                        nc.scalar.copy(dst, ps_xt)

        def emit_ffn_h(blk):
            xT_blk = xT_blks.pop(blk)
            h_sb = h_pool.tile([128, 16, 512], f16, tag="h")
            for ft in range(16):
                ps_h = big_ps.tile([128, 512], f32, tag="big")
                for kt in range(4):
                    nc.tensor.matmul(
                        ps_h,
                        lhsT=w1_sb[:, kt, ft * 128:(ft + 1) * 128],
                        rhs=xT_blk[:, kt, :],
                        start=(kt == 0), stop=(kt == 3),
                    )
                nc.scalar.activation(h_sb[:, ft, :], ps_h, AF.Relu,
                                     bias=b1T_sb[:, ft:ft + 1])
            h_blks[blk] = h_sb

        def newton_rstd(var_ap, den_ap, n, tagp, iters=4):
            """rstd = 1/sqrt(var) on DVE: den-seeded Newton sqrt.
            In the fast path s2 = den*(core), var = den^2*var_core with
            var_core in a tight range, so y0 = 1.26*den converges in 4
            iterations of y <- 0.5*(y + var/y) to ~1e-6 relative."""
            y = stat_pool.tile([128, n], f32, tag=tagp + "y")
            nc.vector.tensor_scalar(y, den_ap, 1.26, None, ALU.mult)
            for _ in range(iters):
                r = stat_pool.tile([128, n], f32, tag=tagp + "r")
                nc.vector.reciprocal(r, y)
                u = stat_pool.tile([128, n], f32, tag=tagp + "u")
                nc.vector.tensor_mul(u, var_ap, r)
                t = stat_pool.tile([128, n], f32, tag=tagp + "t")
                nc.vector.tensor_add(t, y, u)
                y = stat_pool.tile([128, n], f32, tag=tagp + "z")
                nc.vector.tensor_scalar(y, t, 0.5, None, ALU.mult)
            rstd = stat_pool.tile([128, n], f32, tag=tagp + "o")
            nc.vector.reciprocal(rstd, y)
            return rstd

        def emit_ffn_y(blk):
            h_sb = h_blks.pop(blk)
            # last block: rstd via ScalarE ln/exp (2 one-off table loads that
            # overlap matmuls) so the final vector chain stays short
            last = fast and blk == NBLK - 1
            s2s, mvs = [], []
            for j in range(4):
                p = blk * 4 + j
                ps_y = big_ps.tile([128, 512], f32, tag="big")
                for ft in range(16):
                    nc.tensor.matmul(
                        ps_y,
                        lhsT=h_sb[:, ft, j * 128:(j + 1) * 128],
                        rhs=w2_sb[:, ft, :],
                        start=(ft == 0), stop=(ft == 15 and not apply_b2),
                    )
                if apply_b2:
                    nc.tensor.matmul(ps_y, lhsT=onerow2_sb, rhs=b2row_sb,
                                     start=False, stop=True)
                s2 = res_pool.tile([128, D], f16, tag="s2")
                nc.vector.tensor_add(s2, x_tiles.pop(p), ps_y)
                st6 = stat_pool.tile([128, 6], f32, tag="st6b")
                nc.vector.bn_stats(st6, s2)
                mv = stat_pool.tile([128, 2], f32, tag="mv2")
                nc.vector.bn_aggr(mv, st6)
                s2s.append(s2)
                mvs.append(mv)
            var_blk = stat_pool.tile([128, 4], f32, tag="b2v")
            for i in range(4):
                nc.gpsimd.tensor_copy(var_blk[:, i:i + 1], mvs[i][:, 1:2])
            if fast and not last:
                den_blk = stat_pool.tile([128, 4], f32, tag="b2d")
                for j in range(4):
                    nc.gpsimd.tensor_copy(den_blk[:, j:j + 1],
                                          den_tiles.pop(blk * 4 + j))
                rstd_blk = newton_rstd(var_blk, den_blk, 4, "bk")
            else:
                if fast:
                    for j in range(4):
                        den_tiles.pop(blk * 4 + j)
                lnv_blk = stat_pool.tile([128, 4], f32, tag="b2l")
                nc.scalar.activation(lnv_blk, var_blk, AF.Ln, bias=eps_sb)
                rstd_blk = stat_pool.tile([128, 4], f32, tag="b2r")
                nc.scalar.activation(rstd_blk, lnv_blk, AF.Exp, scale=-0.5)
            for j in range(4):
                p = blk * 4 + j
                o_sb = io_pool.tile([128, D], f16, tag="o")
                nc.vector.tensor_scalar(o_sb, s2s[j], mvs[j][:, 0:1],
                                        rstd_blk[:, j:j + 1],
                                        ALU.subtract, ALU.mult)
                if apply_ln2g:
                    nc.vector.tensor_mul(o_sb, o_sb, g2_sb)
                if apply_ln2b:
                    nc.vector.tensor_add(o_sb, o_sb, be2_sb)
                nc.sync.dma_start(out_d[p * 128:(p + 1) * 128, :], o_sb)

        for p in range(NPAIR + 12):
            if p < NPAIR:
                emit_sim(p)
            if 1 <= p <= NPAIR:
                emit_av(p - 1)
            if not fast and p >= 5 and (p - 5) % 4 == 0 and (p - 5) // 4 < NBLK:
                ln_compute((p - 5) // 4)
            if p >= 8 and (p - 8) % 4 == 0 and (p - 8) // 4 < NBLK:
                emit_transposes((p - 8) // 4)
            if p >= 10 and (p - 10) % 4 == 0 and (p - 10) // 4 < NBLK:
                emit_ffn_h((p - 10) // 4)
            if fast and p == 35:
                # preload the natural_log_exp ACT table off the tail critical
                # path; reading h(6) anchors it after the last attention exp
                lnwarm = stat_pool.tile([128, 1], f32, tag="lnwarm")
                nc.scalar.activation(lnwarm, h_blks[6][:, 0, 0:1], AF.Ln,
                                     bias=eps_sb)
            if p >= 11 and (p - 11) % 4 == 0 and (p - 11) // 4 < NBLK:
                emit_ffn_y((p - 11) // 4)

    nc.compile()
    return nc


def _get_program(key):
    if key not in _cache:
        _cache[key] = _build(*key)
    return _cache[key]


last_exec_ns = None


def _install_ntff_hook():
    """NTFF profiling hook for axon (normally installed via antenv.axon_hooks)."""
    import sys, types
    if 'antenv.axon_hooks' in sys.modules:
        return
    mod = types.ModuleType('antenv.axon_hooks')
    _h = [None]
    mod.set_axon_ntff_profile_hook = lambda h: _h.__setitem__(0, h)
    mod.get_axon_ntff_profile_hook = lambda: _h[0]
    sys.modules['antenv.axon_hooks'] = mod
    import antenv
    antenv.axon_hooks = mod
    try:
        from trn_agent_boot.trn_boot import _ntff_profile_via_ctypes
        mod.set_axon_ntff_profile_hook(
            _ntff_profile_via_ctypes('/opt/axon/libaxon_pjrt.so'))
    except Exception:
        pass


def kernel(src, mask, Wq, bq, Wk, bk, Wv, bv, ln1_g, ln1_b,
           W1, b1, W2, b2, ln2_g, ln2_b):
    global last_exec_ns
    src = np.asarray(src, np.float32)
    if not bool(np.asarray(mask).all()):
        raise NotImplementedError("only all-true mask supported")

    key = (bool(np.any(bv)), bool(np.any(b2)),
           not bool(np.all(ln1_g == 1)), bool(np.any(ln1_b)),
           not bool(np.all(ln2_g == 1)), bool(np.any(ln2_b)),
           bool(np.any(b1)), bool(np.any(bq)) or bool(np.any(bk)))
    nc = _get_program(key)
    apply_bv, apply_b2, a_g1, a_b1, a_g2, a_b2, _ab1, apply_bqk = key

    qi = np.arange(128)
    wA = np.where(qi >= 64, NEG, 0.0).astype(_F16).reshape(1, 128)
    wB = np.where(qi < 64, NEG, 0.0).astype(_F16).reshape(1, 128)
    wfull = np.full((1, 128), NEG, _F16)
    uA = (qi < 64).astype(_F16).reshape(1, 128)
    uB = (qi >= 64).astype(_F16).reshape(1, 128)

    shared = {
        "wv": Wv.astype(_F16),
        "w1": W1.astype(_F16),
        "b1T": np.asarray(b1, np.float32).reshape(16, 128).T.copy(),
        "w2": W2.astype(_F16),
        "ident": np.eye(128, dtype=_F16),
        "uA": uA, "uB": uB, "wA": wA, "wB": wB,
        "onerow": np.ones((1, 128), _F16),
    }
    if apply_bqk:
        shared["wq"] = Wq.astype(_F16)
        shared["wk"] = Wk.astype(_F16)
        shared["bqT"] = np.asarray(bq, np.float32).reshape(4, 128).T.copy()
        shared["bkT"] = np.asarray(bk, np.float32).reshape(4, 128).T.copy()
    else:
        # merged QK: sim = (src (Wq Wk^T)) src^T
        M = np.asarray(Wq, np.float32) @ np.asarray(Wk, np.float32).T
        shared["wq"] = M.astype(_F16)
    if apply_bv:
        shared["bvrow"] = np.asarray(bv, np.float32).reshape(1, D).astype(_F16)
    if apply_b2:
        shared["b2row"] = np.asarray(b2, np.float32).reshape(1, D).astype(_F16)
    if a_g1:
        shared["g1"] = np.tile(np.asarray(ln1_g, _F16).reshape(1, D), (128, 1))
    if a_b1:
        shared["be1"] = np.tile(np.asarray(ln1_b, _F16).reshape(1, D), (128, 1))
    if a_g2:
        shared["g2"] = np.tile(np.asarray(ln2_g, _F16).reshape(1, D), (128, 1))
    if a_b2:
        shared["be2"] = np.tile(np.asarray(ln2_b, _F16).reshape(1, D), (128, 1))

    in_maps = []
    for c in range(8):
        b, h = divmod(c, 2)
        start = h * T - H
        ext = np.zeros((TEXT, D), np.float32)
        lo, hi = max(start, 0), min(start + TEXT, N)
        ext[lo - start: hi - start] = src[b, lo:hi]
        m = dict(shared)
        m["srcT"] = np.ascontiguousarray(ext.T).astype(_F16)
        m["src"] = np.ascontiguousarray(src[b, h * T:(h + 1) * T]).astype(_F16)
        m["wA0"] = wfull if h == 0 else wA
        m["wB31"] = wfull if h == 1 else wB
        in_maps.append(m)

    from concourse.bass_utils import run_bass_kernel_spmd
    trace = bool(os.environ.get("KERNEL_TRACE"))
    if trace:
        _install_ntff_hook()
    res = run_bass_kernel_spmd(nc, in_maps, core_ids=list(range(8)), trace=trace)
    if trace:
        last_exec_ns = res.exec_time_ns

    out = np.empty((B, N, D), np.float32)
    for c in range(8):
        b, h = divmod(c, 2)
        out[b, h * T:(h + 1) * T] = res.results[c]["out"].astype(np.float32)
    return out
